# revision 1
# baseline (speedup 1.0000x reference)
"""Trainium2 Bass kernel for nn_CopyLayer (pointer-generator copy layer).

Computes, for inputs of fixed shape (hardcoded per the problem spec):
    gate  = sigmoid(ds @ w_copy + b_copy)              # N x L x 1
    probs = softmax(ds @ w_gen + b_gen, axis=-1)       # N x L x V
    copy  = scatter_add(attn_copy over src_token_ids)  # N x L x V
    out   = gate * probs + (1 - gate) * copy

Strategy: tensor-parallel over the vocab dim V (8 cores x 4000 columns).
Each core computes its logits shard with a bf16 tiled matmul, exponentiates
on the scalar engine (with fused per-row partial sums), all-reduces the
softmax denominators (2 KB per batch chunk), and combines with the copy
distribution.  The scatter is reformulated as a small dense matmul:
host-side we compact, per (batch, core), the <=128 source positions whose
token ids land in this core's vocab shard, and build the matching one-hot
matrix; `attn_w @ onehot` then reproduces scatter-with-duplicate-accumulation
exactly.  The (1-gate) weighting is folded into attn_w and gate/Z into the
final fused DVE op, so the combine is a single pass over the output.
"""

import numpy as np
import ml_dtypes

import concourse.bass as bass
import concourse.bacc as bacc
import concourse.mybir as mybir
import concourse.tile as tile
from concourse.bass_utils import run_bass_kernel_spmd

N, L, S, D, V = 4, 512, 512, 1024, 32000
NCORES = 8
VS = V // NCORES          # 4000 vocab columns per core
NL = N * L                # 2048 rows
RT = NL // 128            # 16 row tiles
KT = D // 128             # 8 contraction tiles
CTW = 500                 # column tile width (<=512, fits one PSUM bank)
CT = VS // CTW            # 8 column tiles
RPN = L // 128            # 4 row tiles per batch element

BF16 = mybir.dt.bfloat16
F16 = mybir.dt.float16
F32 = mybir.dt.float32

_NC_CACHE = {}


def _build(cb: int, has_bgen: bool):
    """Build the SPMD Bass program. cb = number of 128-row compaction blocks
    for the scatter matmul; has_bgen = add the b_gen bias via a K=1 matmul."""
    nc = bacc.Bacc("TRN2", target_bir_lowering=False, debug=False,
                   num_devices=NCORES)

    ds_pre = nc.dram_tensor("ds_pre", [RT, 128, D], BF16, kind="ExternalInput")
    w_pre = nc.dram_tensor("w_pre", [KT, 128, VS], BF16, kind="ExternalInput")
    oh_pre = nc.dram_tensor("oh_pre", [N * cb, 128, VS], BF16,
                            kind="ExternalInput")
    at_pre = nc.dram_tensor("at_pre", [N * cb, 128, L], BF16,
                            kind="ExternalInput")
    gate_pre = nc.dram_tensor("gate_pre", [128, RT], F32, kind="ExternalInput")
    if has_bgen:
        bgen_pre = nc.dram_tensor("bgen_pre", [1, VS], BF16,
                                  kind="ExternalInput")
    out_dram = nc.dram_tensor("out", [NL, VS], F32, kind="ExternalOutput")

    groups = [list(range(NCORES))]

    with tile.TileContext(nc) as tc:
        with (
            tc.tile_pool(name="wpool", bufs=1) as wpool,
            tc.tile_pool(name="dspool", bufs=3) as dspool,
            tc.tile_pool(name="ohpool", bufs=2) as ohpool,
            tc.tile_pool(name="exppool", bufs=8) as exppool,
            tc.tile_pool(name="outpool", bufs=2) as outpool,
            tc.tile_pool(name="smpool", bufs=16) as smpool,
            tc.tile_pool(name="lps", bufs=4, space="PSUM") as lpsp,
            tc.tile_pool(name="cps", bufs=4, space="PSUM") as cpsp,
            tc.tile_pool(name="dram", bufs=2, space="DRAM") as drp,
        ):
            # --- resident loads -------------------------------------------
            w_sb = wpool.tile([128, KT * VS], BF16, tag="w")
            for kt in range(KT):
                nc.sync.dma_start(w_sb[:, kt * VS:(kt + 1) * VS], w_pre[kt])
            gate_sb = smpool.tile([128, RT], F32, tag="gate")
            nc.sync.dma_start(gate_sb[:], gate_pre[:])
            if has_bgen:
                bgen_sb = smpool.tile([1, VS], BF16, tag="bgen")
                nc.sync.dma_start(bgen_sb[:], bgen_pre[:])
                ones_sb = smpool.tile([1, 128], BF16, tag="ones")
                nc.vector.memset(ones_sb[:], 1.0)

            # --- dummy collective: absorbs CC channel setup / core skew ---
            zero_sb = smpool.tile([128, 1], F32, tag="zero")
            nc.vector.memset(zero_sb[:], 0.0)
            dum_in = drp.tile([128, 1], F32, tag="dumin")
            dum_out = drp.tile([128, 1], F32, tag="dumout")
            nc.gpsimd.dma_start(dum_in[:], zero_sb[:])
            nc.gpsimd.collective_compute(
                "AllReduce", mybir.AluOpType.add, replica_groups=groups,
                ins=[dum_in.opt()], outs=[dum_out.opt()])

            for n in range(N):
                oh_ts, at_ts = [], []
                for b in range(cb):
                    oh_t = ohpool.tile([128, VS], BF16, tag="oh")
                    at_t = ohpool.tile([128, L], BF16, tag="at")
                    nc.sync.dma_start(oh_t[:], oh_pre[n * cb + b])
                    nc.sync.dma_start(at_t[:], at_pre[n * cb + b])
                    oh_ts.append(oh_t)
                    at_ts.append(at_t)

                zc = smpool.tile([128, RPN], F32, tag="zc")
                exp_ts = []
                for j in range(RPN):
                    rt = n * RPN + j
                    ds_t = dspool.tile([128, D], BF16, tag="ds")
                    nc.sync.dma_start(ds_t[:], ds_pre[rt])
                    exp_t = exppool.tile([128, VS], F16, tag="exp")
                    zp = smpool.tile([128, CT], F32, tag="zp")
                    for ct in range(CT):
                        ps = lpsp.tile([128, CTW], F32, tag="lps")
                        for kt in range(KT):
                            nc.tensor.matmul(
                                ps[:],
                                ds_t[:, kt * 128:(kt + 1) * 128],
                                w_sb[:, kt * VS + ct * CTW:
                                     kt * VS + (ct + 1) * CTW],
                                start=(kt == 0),
                                stop=(kt == KT - 1 and not has_bgen),
                            )
                        if has_bgen:
                            nc.tensor.matmul(
                                ps[:], ones_sb[:],
                                bgen_sb[:, ct * CTW:(ct + 1) * CTW],
                                start=False, stop=True)
                        nc.scalar.activation(
                            exp_t[:, ct * CTW:(ct + 1) * CTW], ps[:],
                            mybir.ActivationFunctionType.Exp,
                            accum_out=zp[:, ct:ct + 1])
                    nc.vector.tensor_reduce(
                        zc[:, j:j + 1], zp[:], axis=mybir.AxisListType.X,
                        op=mybir.AluOpType.add)
                    exp_ts.append(exp_t)

                # --- all-reduce the softmax denominators for this n -------
                z_in = drp.tile([128, RPN], F32, tag="zin")
                z_out = drp.tile([128, RPN], F32, tag="zout")
                nc.gpsimd.dma_start(z_in[:], zc[:])
                nc.gpsimd.collective_compute(
                    "AllReduce", mybir.AluOpType.add, replica_groups=groups,
                    ins=[z_in.opt()], outs=[z_out.opt()])
                zall = smpool.tile([128, RPN], F32, tag="za")
                nc.sync.dma_start(zall[:], z_out[:])

                for j in range(RPN):
                    rt = n * RPN + j
                    zrec = smpool.tile([128, 1], F32, tag="zr")
                    nc.vector.reciprocal(zrec[:], zall[:, j:j + 1])
                    s_t = smpool.tile([128, 1], F32, tag="s")
                    nc.vector.tensor_mul(s_t[:], zrec[:],
                                         gate_sb[:, rt:rt + 1])
                    out_t = outpool.tile([128, VS], F32, tag="out")
                    for ct in range(CT):
                        cps = cpsp.tile([128, CTW], F32, tag="cps")
                        for b in range(cb):
                            nc.tensor.matmul(
                                cps[:],
                                at_ts[b][:, j * 128:(j + 1) * 128],
                                oh_ts[b][:, ct * CTW:(ct + 1) * CTW],
                                start=(b == 0), stop=(b == cb - 1))
                        # out = exp * (gate/Z) + copy
                        nc.vector.scalar_tensor_tensor(
                            out_t[:, ct * CTW:(ct + 1) * CTW],
                            exp_ts[j][:, ct * CTW:(ct + 1) * CTW],
                            s_t[:, 0:1], cps[:],
                            op0=mybir.AluOpType.mult,
                            op1=mybir.AluOpType.add)
                    nc.sync.dma_start(out_dram[rt * 128:(rt + 1) * 128, :],
                                      out_t[:])
    nc.compile()
    return nc


def kernel(decoder_states, attn_copy, src_token_ids, w_copy, b_copy,
           w_gen, b_gen):
    ds2 = np.asarray(decoder_states, dtype=np.float32).reshape(NL, D)
    attn = np.asarray(attn_copy, dtype=np.float32)
    ids = np.asarray(src_token_ids).reshape(N, S).astype(np.int64)
    w_copy = np.asarray(w_copy, dtype=np.float32).reshape(D)
    b_copy = float(np.asarray(b_copy).reshape(-1)[0])
    w_gen = np.asarray(w_gen, dtype=np.float32)
    b_gen = np.asarray(b_gen, dtype=np.float32).reshape(V)

    # gate (a 2M-FLOP matvec) on host; (1-gate) is folded into attn below.
    z = ds2.astype(np.float64) @ w_copy.astype(np.float64) + b_copy
    gate = (1.0 / (1.0 + np.exp(-z))).astype(np.float32)          # [NL]

    # PE-friendly layouts -------------------------------------------------
    # ds_pre[rt, p, kt*128+l] = ds2[rt*128+l, kt*128+p]
    ds_pre = np.ascontiguousarray(
        ds2.reshape(RT, 128, KT, 128).transpose(0, 3, 2, 1)
    ).astype(ml_dtypes.bfloat16)
    gate_pre = np.ascontiguousarray(gate.reshape(RT, 128).T)      # [128, RT]
    w_bf = w_gen.astype(ml_dtypes.bfloat16)

    # per-(core, n) compaction of scatter sources -------------------------
    omg = (1.0 - gate).reshape(N, L).astype(np.float32)
    sels = [[np.nonzero((ids[n] >= k * VS) & (ids[n] < (k + 1) * VS))[0]
             for n in range(N)] for k in range(NCORES)]
    cb = max(1, -(-max(len(s) for row in sels for s in row) // 128))
    has_bgen = bool(np.any(b_gen))

    key = (cb, has_bgen)
    if key not in _NC_CACHE:
        _NC_CACHE[key] = _build(cb, has_bgen)
    nc = _NC_CACHE[key]

    in_maps = []
    for k in range(NCORES):
        w_pre = np.ascontiguousarray(
            w_bf[:, k * VS:(k + 1) * VS].reshape(KT, 128, VS))
        oh_pre = np.zeros((N * cb, 128, VS), dtype=ml_dtypes.bfloat16)
        at_pre = np.zeros((N * cb, 128, L), dtype=np.float32)
        for n in range(N):
            sel = sels[k][n]
            ids_rel = ids[n, sel] - k * VS
            aw = attn[n][:, sel] * omg[n][:, None]     # [L, cnt]
            for b in range(cb):
                lo, hi = b * 128, min((b + 1) * 128, len(sel))
                if lo >= hi:
                    break
                cnt = hi - lo
                oh_pre[n * cb + b][np.arange(cnt), ids_rel[lo:hi]] = 1.0
                at_pre[n * cb + b][:cnt, :] = aw[:, lo:hi].T
        im = {
            "ds_pre": ds_pre,
            "w_pre": w_pre,
            "oh_pre": oh_pre,
            "at_pre": at_pre.astype(ml_dtypes.bfloat16),
            "gate_pre": gate_pre,
        }
        if has_bgen:
            im["bgen_pre"] = np.ascontiguousarray(
                b_gen[k * VS:(k + 1) * VS].reshape(1, VS)
            ).astype(ml_dtypes.bfloat16)
        in_maps.append(im)

    res = run_bass_kernel_spmd(nc, in_maps, core_ids=list(range(NCORES)))
    out = np.concatenate([res.results[k]["out"] for k in range(NCORES)],
                         axis=1)
    return out.reshape(N, L, V)


# revision 2
# speedup vs baseline: 1.1088x; 1.1088x over previous
"""Trainium2 Bass kernel for nn_CopyLayer (pointer-generator copy layer).

Computes, for inputs of fixed shape (hardcoded per the problem spec):
    gate  = sigmoid(ds @ w_copy + b_copy)              # N x L x 1
    probs = softmax(ds @ w_gen + b_gen, axis=-1)       # N x L x V
    copy  = scatter_add(attn_copy over src_token_ids)  # N x L x V
    out   = gate * probs + (1 - gate) * copy

Strategy: tensor-parallel over the vocab dim V (8 cores x 4000 columns).
Each core computes its logits shard with a bf16 tiled matmul, exponentiates
on the scalar engine (with fused per-row partial sums), all-reduces the
softmax denominators (1 KB per 256-row chunk), and combines with the copy
distribution.  The scatter is reformulated as a small dense matmul:
host-side we compact, per (batch, core), the <=128 source positions whose
token ids land in this core's vocab shard, and build the matching one-hot
matrix; `attn_w @ onehot` then reproduces scatter-with-duplicate-accumulation
exactly.  The (1-gate) weighting is folded into attn_w and gate/Z into the
final fused DVE op, so the combine is a single pass over the output.

The weight shard is laid out column-tile-major so the first matmul only
needs 1 MB of weights on chip; the rest streams in under compute.
"""

import numpy as np
import ml_dtypes

import concourse.bass as bass
import concourse.bacc as bacc
import concourse.mybir as mybir
import concourse.tile as tile
from concourse.bass_utils import run_bass_kernel_spmd

N, L, S, D, V = 4, 512, 512, 1024, 32000
NCORES = 8
VS = V // NCORES          # 4000 vocab columns per core
NL = N * L                # 2048 rows
RT = NL // 128            # 16 row tiles
KT = D // 128             # 8 contraction tiles
CTW = 500                 # column tile width (<=512, fits one PSUM bank)
CT = VS // CTW            # 8 column tiles
RPC = 2                   # row tiles per all-reduce chunk
NCHUNK = RT // RPC        # 8 chunks
RPN = L // 128            # 4 row tiles per batch element

BF16 = mybir.dt.bfloat16
F16 = mybir.dt.float16
F32 = mybir.dt.float32

_NC_CACHE = {}


def _build(cb: int, has_bgen: bool):
    """Build the SPMD Bass program. cb = number of 128-row compaction blocks
    for the scatter matmul; has_bgen = add the b_gen bias via a K=1 matmul."""
    nc = bacc.Bacc("TRN2", target_bir_lowering=False, debug=False,
                   num_devices=NCORES)

    ds_pre = nc.dram_tensor("ds_pre", [RT, 128, D], BF16, kind="ExternalInput")
    # ct-major weight layout: w_pre[ct, p, kt*CTW + c]
    w_pre = nc.dram_tensor("w_pre", [CT, 128, KT * CTW], BF16,
                           kind="ExternalInput")
    oh_pre = nc.dram_tensor("oh_pre", [N * cb, 128, VS], BF16,
                            kind="ExternalInput")
    at_pre = nc.dram_tensor("at_pre", [N * cb, 128, L], BF16,
                            kind="ExternalInput")
    gate_pre = nc.dram_tensor("gate_pre", [128, RT], F32, kind="ExternalInput")
    if has_bgen:
        bgen_pre = nc.dram_tensor("bgen_pre", [1, VS], BF16,
                                  kind="ExternalInput")
    out_dram = nc.dram_tensor("out", [NL, VS], F32, kind="ExternalOutput")

    groups = [list(range(NCORES))]

    with tile.TileContext(nc) as tc:
        with (
            tc.tile_pool(name="wpool", bufs=1) as wpool,
            tc.tile_pool(name="dspool", bufs=4) as dspool,
            tc.tile_pool(name="ohpool", bufs=2) as ohpool,
            tc.tile_pool(name="exppool", bufs=6) as exppool,
            tc.tile_pool(name="outpool", bufs=3) as outpool,
            tc.tile_pool(name="smpool", bufs=16) as smpool,
            tc.tile_pool(name="lps", bufs=4, space="PSUM") as lpsp,
            tc.tile_pool(name="cps", bufs=4, space="PSUM") as cpsp,
            tc.tile_pool(name="dram", bufs=2, space="DRAM") as drp,
        ):
            # --- resident loads, most-urgent first ------------------------
            w_sb = wpool.tile([128, CT * KT * CTW], BF16, tag="w")

            def w_off(ct, kt):
                return ct * (KT * CTW) + kt * CTW

            nc.sync.dma_start(w_sb[:, 0 * KT * CTW:1 * KT * CTW], w_pre[0])
            ds_t0 = dspool.tile([128, D], BF16, tag="ds")
            nc.sync.dma_start(ds_t0[:], ds_pre[0])
            for ct in range(1, CT):
                nc.sync.dma_start(
                    w_sb[:, ct * KT * CTW:(ct + 1) * KT * CTW], w_pre[ct])
            gate_sb = smpool.tile([128, RT], F32, tag="gate")
            nc.sync.dma_start(gate_sb[:], gate_pre[:])
            if has_bgen:
                bgen_sb = smpool.tile([1, VS], BF16, tag="bgen")
                nc.sync.dma_start(bgen_sb[:], bgen_pre[:])
                ones_sb = smpool.tile([1, 128], BF16, tag="ones")
                nc.vector.memset(ones_sb[:], 1.0)

            # --- dummy collective: absorbs CC channel setup / core skew ---
            zero_sb = smpool.tile([128, 1], F32, tag="zero")
            nc.vector.memset(zero_sb[:], 0.0)
            dum_in = drp.tile([128, 1], F32, tag="dumin")
            dum_out = drp.tile([128, 1], F32, tag="dumout")
            nc.gpsimd.dma_start(dum_in[:], zero_sb[:])
            nc.gpsimd.collective_compute(
                "AllReduce", mybir.AluOpType.add, replica_groups=groups,
                ins=[dum_in.opt()], outs=[dum_out.opt()])

            oh_ts, at_ts = {}, {}
            for c in range(NCHUNK):
                n = (c * RPC) // RPN
                if (c * RPC) % RPN == 0:
                    oh_ts[n], at_ts[n] = [], []
                    for b in range(cb):
                        oh_t = ohpool.tile([128, VS], BF16, tag="oh")
                        at_t = ohpool.tile([128, L], BF16, tag="at")
                        nc.sync.dma_start(oh_t[:], oh_pre[n * cb + b])
                        nc.sync.dma_start(at_t[:], at_pre[n * cb + b])
                        oh_ts[n].append(oh_t)
                        at_ts[n].append(at_t)

                zc = smpool.tile([128, RPC], F32, tag="zc")
                exp_ts = []
                for j in range(RPC):
                    rt = c * RPC + j
                    if rt == 0:
                        ds_t = ds_t0
                    else:
                        ds_t = dspool.tile([128, D], BF16, tag="ds")
                        nc.sync.dma_start(ds_t[:], ds_pre[rt])
                    exp_t = exppool.tile([128, VS], F16, tag="exp")
                    zp = smpool.tile([128, CT], F32, tag="zp")
                    for ct in range(CT):
                        ps = lpsp.tile([128, CTW], F32, tag="lps")
                        for kt in range(KT):
                            nc.tensor.matmul(
                                ps[:],
                                ds_t[:, kt * 128:(kt + 1) * 128],
                                w_sb[:, w_off(ct, kt):w_off(ct, kt) + CTW],
                                start=(kt == 0),
                                stop=(kt == KT - 1 and not has_bgen),
                            )
                        if has_bgen:
                            nc.tensor.matmul(
                                ps[:], ones_sb[:],
                                bgen_sb[:, ct * CTW:(ct + 1) * CTW],
                                start=False, stop=True)
                        nc.scalar.activation(
                            exp_t[:, ct * CTW:(ct + 1) * CTW], ps[:],
                            mybir.ActivationFunctionType.Exp,
                            accum_out=zp[:, ct:ct + 1])
                    nc.vector.tensor_reduce(
                        zc[:, j:j + 1], zp[:], axis=mybir.AxisListType.X,
                        op=mybir.AluOpType.add)
                    exp_ts.append(exp_t)

                # --- all-reduce the softmax denominators for this chunk ---
                z_in = drp.tile([128, RPC], F32, tag="zin")
                z_out = drp.tile([128, RPC], F32, tag="zout")
                nc.gpsimd.dma_start(z_in[:], zc[:])
                nc.gpsimd.collective_compute(
                    "AllReduce", mybir.AluOpType.add, replica_groups=groups,
                    ins=[z_in.opt()], outs=[z_out.opt()])
                zall = smpool.tile([128, RPC], F32, tag="za")
                nc.sync.dma_start(zall[:], z_out[:])

                for j in range(RPC):
                    rt = c * RPC + j
                    jn = rt % RPN        # row-tile index within batch n
                    zrec = smpool.tile([128, 1], F32, tag="zr")
                    nc.vector.reciprocal(zrec[:], zall[:, j:j + 1])
                    s_t = smpool.tile([128, 1], F32, tag="s")
                    nc.vector.tensor_mul(s_t[:], zrec[:],
                                         gate_sb[:, rt:rt + 1])
                    out_t = outpool.tile([128, VS], F32, tag="out")
                    for ct in range(CT):
                        cps = cpsp.tile([128, CTW], F32, tag="cps")
                        for b in range(cb):
                            nc.tensor.matmul(
                                cps[:],
                                at_ts[n][b][:, jn * 128:(jn + 1) * 128],
                                oh_ts[n][b][:, ct * CTW:(ct + 1) * CTW],
                                start=(b == 0), stop=(b == cb - 1))
                        # out = exp * (gate/Z) + copy
                        nc.vector.scalar_tensor_tensor(
                            out_t[:, ct * CTW:(ct + 1) * CTW],
                            exp_ts[j][:, ct * CTW:(ct + 1) * CTW],
                            s_t[:, 0:1], cps[:],
                            op0=mybir.AluOpType.mult,
                            op1=mybir.AluOpType.add)
                    nc.sync.dma_start(out_dram[rt * 128:(rt + 1) * 128, :],
                                      out_t[:])
    nc.compile()
    return nc


def kernel(decoder_states, attn_copy, src_token_ids, w_copy, b_copy,
           w_gen, b_gen):
    ds2 = np.asarray(decoder_states, dtype=np.float32).reshape(NL, D)
    attn = np.asarray(attn_copy, dtype=np.float32)
    ids = np.asarray(src_token_ids).reshape(N, S).astype(np.int64)
    w_copy = np.asarray(w_copy, dtype=np.float32).reshape(D)
    b_copy = float(np.asarray(b_copy).reshape(-1)[0])
    w_gen = np.asarray(w_gen, dtype=np.float32)
    b_gen = np.asarray(b_gen, dtype=np.float32).reshape(V)

    # gate (a 2M-FLOP matvec) on host; (1-gate) is folded into attn below.
    z = ds2.astype(np.float64) @ w_copy.astype(np.float64) + b_copy
    gate = (1.0 / (1.0 + np.exp(-z))).astype(np.float32)          # [NL]

    # PE-friendly layouts -------------------------------------------------
    # ds_pre[rt, p, kt*128+l] = ds2[rt*128+l, kt*128+p]
    ds_pre = np.ascontiguousarray(
        ds2.reshape(RT, 128, KT, 128).transpose(0, 3, 2, 1)
    ).astype(ml_dtypes.bfloat16)
    gate_pre = np.ascontiguousarray(gate.reshape(RT, 128).T)      # [128, RT]
    w_bf = w_gen.astype(ml_dtypes.bfloat16)

    # per-(core, n) compaction of scatter sources -------------------------
    omg = (1.0 - gate).reshape(N, L).astype(np.float32)
    sels = [[np.nonzero((ids[n] >= k * VS) & (ids[n] < (k + 1) * VS))[0]
             for n in range(N)] for k in range(NCORES)]
    cb = max(1, -(-max(len(s) for row in sels for s in row) // 128))
    has_bgen = bool(np.any(b_gen))

    key = (cb, has_bgen)
    if key not in _NC_CACHE:
        _NC_CACHE[key] = _build(cb, has_bgen)
    nc = _NC_CACHE[key]

    in_maps = []
    for k in range(NCORES):
        # w_pre[ct, p, kt*CTW + c] = w_gen[kt*128 + p, k*VS + ct*CTW + c]
        wk = w_bf[:, k * VS:(k + 1) * VS].reshape(KT, 128, CT, CTW)
        w_pre = np.ascontiguousarray(wk.transpose(2, 1, 0, 3).reshape(
            CT, 128, KT * CTW))
        oh_pre = np.zeros((N * cb, 128, VS), dtype=ml_dtypes.bfloat16)
        at_pre = np.zeros((N * cb, 128, L), dtype=np.float32)
        for n in range(N):
            sel = sels[k][n]
            ids_rel = ids[n, sel] - k * VS
            aw = attn[n][:, sel] * omg[n][:, None]     # [L, cnt]
            for b in range(cb):
                lo, hi = b * 128, min((b + 1) * 128, len(sel))
                if lo >= hi:
                    break
                cnt = hi - lo
                oh_pre[n * cb + b][np.arange(cnt), ids_rel[lo:hi]] = 1.0
                at_pre[n * cb + b][:cnt, :] = aw[:, lo:hi].T
        im = {
            "ds_pre": ds_pre,
            "w_pre": w_pre,
            "oh_pre": oh_pre,
            "at_pre": at_pre.astype(ml_dtypes.bfloat16),
            "gate_pre": gate_pre,
        }
        if has_bgen:
            im["bgen_pre"] = np.ascontiguousarray(
                b_gen[k * VS:(k + 1) * VS].reshape(1, VS)
            ).astype(ml_dtypes.bfloat16)
        in_maps.append(im)

    res = run_bass_kernel_spmd(nc, in_maps, core_ids=list(range(NCORES)))
    out = np.concatenate([res.results[k]["out"] for k in range(NCORES)],
                         axis=1)
    return out.reshape(N, L, V)
